# revision 11
# baseline (speedup 1.0000x reference)
"""DipoleMomentDecoder Trainium2 kernel (8-core SPMD, full I/O).

Strategy
--------
Shard by graph: core k owns graphs [k*G, (k+1)*G), G = ceil(B/8).  batch_index
is sorted, so each core gets a contiguous token range (padded to Nmax tokens).

On-chip layout is feature-major ([features(partitions), tokens(free)]): the two
gate blocks chain matmuls without any transposes.  Host pre-transposes/casts
the big activations (vector, scaler) to bf16 feature-major; matmuls run bf16
at full PE rate (fp32 matmul is 4x slower on TRN2).

ACT table sets: Sqrt and Silu live in different table sets (~2.7us per switch),
so the kernel is phase-batched globally: P1 einsum0+squares (all tiles), P2
sqrt, P3 MLP0+silu, P4 einsum1+squares (+ token-major vec_w1 mini-matmuls),
P5 sqrt, P6 MLP1+silu (+ token-major q1/gate1 mini-matmuls), P7 node_mu +
one-hot segment-sum matmul, P8 final norm.  Square/Copy/Identity are in every
set so only 5 table loads happen.

Segment sum is data-driven (one-hot built on device from graph ids via
is_equal) so the single SPMD program works for every core; the final [G,1]
norms are computed on device, the host only concatenates the 8 slices.
"""

import sys

for _p in ("/opt/trn_rl_repo", "/root/.axon_site/_ro/trn_rl_repo"):
    if _p not in sys.path:
        sys.path.insert(0, _p)

import numpy as np
from ml_dtypes import bfloat16

import concourse.bacc as bacc
import concourse.tile as tile
from concourse import mybir
from concourse.bass_utils import run_bass_kernel_spmd

F = mybir.dt
AF = mybir.ActivationFunctionType
ALU = mybir.AluOpType
X = mybir.AxisListType.X

IN_F = 256
HID = 128
N_CORES = 8
TILE = 512  # tokens per tile
PCH = 128  # tokens per partition-chunk

_cache = {}


def _repack_k(w):
    """[K, O] -> [128, (K//128)*O]; cols [k*O + m] = w[k*128 + p, m]."""
    k, o = w.shape
    kc = k // 128
    return np.ascontiguousarray(
        w.reshape(kc, 128, o).transpose(1, 0, 2).reshape(128, kc * o)
    )


def _build(Nmax, Gmax):
    T = Nmax // TILE
    NCH = Nmax // PCH
    nc = bacc.Bacc("TRN2", target_bir_lowering=False, debug=False,
                   num_devices=N_CORES)
    bf, f32 = F.bfloat16, F.float32

    # -------- DRAM I/O --------
    vT = nc.dram_tensor("vT", [3, IN_F, Nmax], bf, kind="ExternalInput")
    sT = nc.dram_tensor("sT", [IN_F, Nmax], bf, kind="ExternalInput")
    mcv = nc.dram_tensor("mcv", [Nmax, 3], f32, kind="ExternalInput")
    ids = nc.dram_tensor("ids", [Nmax], f32, kind="ExternalInput")
    w0_d = nc.dram_tensor("w0", [128, 2 * 384], bf, kind="ExternalInput")
    m1w_d = nc.dram_tensor("m1w", [128, 4 * 512], bf, kind="ExternalInput")
    m2w_d = nc.dram_tensor("m2w", [128, 4 * 256], bf, kind="ExternalInput")
    w1g_d = nc.dram_tensor("w1g", [128, 129], bf, kind="ExternalInput")
    m1wg_d = nc.dram_tensor("m1wg", [128, 2 * 256], bf, kind="ExternalInput")
    m2wg_d = nc.dram_tensor("m2wg", [128, 2 * 2], bf, kind="ExternalInput")
    m1b_d = nc.dram_tensor("m1b", [128, 4], f32, kind="ExternalInput")
    m2b_d = nc.dram_tensor("m2b", [128, 2], f32, kind="ExternalInput")
    m1bg_d = nc.dram_tensor("m1bg", [128, 2], f32, kind="ExternalInput")
    brow2_d = nc.dram_tensor("brow2", [128, 2], f32, kind="ExternalInput")
    iota_d = nc.dram_tensor("iota", [128, Gmax], f32, kind="ExternalInput")
    out_d = nc.dram_tensor("out", [Gmax, 1], f32, kind="ExternalOutput")

    vT_r = vT[:].rearrange("c (k p) n -> p c k n", p=128)  # [128,3,2,Nmax]
    sT_r = sT[:].rearrange("(k p) n -> p k n", p=128)      # [128,2,Nmax]
    mcv_r = mcv[:].rearrange("(u p) c -> p u c", p=128)    # [128,NCH,3]
    ids_r = ids[:].rearrange("(u p) -> p u", p=128)        # [128,NCH]

    with tile.TileContext(nc) as tc:
        # ---- constants / weights (resident) ----
        wp = tc.alloc_tile_pool(name="wp", bufs=1)
        w0_sb = wp.tile([128, 2 * 384], bf)
        m1w_sb = wp.tile([128, 4 * 512], bf)
        m2w_sb = wp.tile([128, 4 * 256], bf)
        w1g_sb = wp.tile([128, 129], bf)
        m1wg_sb = wp.tile([128, 2 * 256], bf)
        m2wg_sb = wp.tile([128, 2 * 2], bf)
        m1b_sb = wp.tile([128, 4], f32)
        m2b_sb = wp.tile([128, 2], f32)
        m1bg_sb = wp.tile([128, 2], f32)
        brow2_sb = wp.tile([128, 2], f32)
        iota_sb = wp.tile([128, Gmax], f32)
        for sb, d in [(w0_sb, w0_d), (m1w_sb, m1w_d), (m2w_sb, m2w_d),
                      (w1g_sb, w1g_d), (m1wg_sb, m1wg_d), (m2wg_sb, m2wg_d),
                      (m1b_sb, m1b_d), (m2b_sb, m2b_d), (m1bg_sb, m1bg_d),
                      (brow2_sb, brow2_d), (iota_sb, iota_d)]:
            nc.sync.dma_start(sb[:], d[:])

        # ---- slabs that live (almost) the whole kernel ----
        slabA = tc.alloc_tile_pool(name="slabA", bufs=1)
        q_sb = slabA.tile([128, Nmax], bf)             # silu(gate0 s_out)
        vw1T_sb = slabA.tile([128, NCH, 3], bf)        # token-major vec_w1
        qg1T_sb = slabA.tile([128, NCH, 2], f32)       # token-major (q1,gate1)
        vecw0_sb = slabA.tile([128, 3, Nmax], bf)      # vec_w g0; becomes v1

        # ---- per-tile working pools (small, persistent) ----
        pv = tc.alloc_tile_pool(name="pv", bufs=2)     # vT tiles
        psq = tc.alloc_tile_pool(name="psq", bufs=2)   # squares
        ph = tc.alloc_tile_pool(name="ph", bufs=2)     # h / h1 / gate
        psm = tc.alloc_tile_pool(name="psm", bufs=4)   # small P7 tiles

        # ================= P1: gate0 einsum + norm^2 =================
        with tc.tile_pool(name="slabC", bufs=1) as slabC:
            sT_sb = slabC.tile([128, 2, Nmax], bf)
            n0sq = slabC.tile([128, 2, Nmax], bf)      # norm^2 then norm (inplace)
            psA = tc.alloc_tile_pool(name="psA", bufs=2, space="PSUM")
            for t in range(T):
                ts = slice(t * TILE, (t + 1) * TILE)
                vt = pv.tile([128, 3, 2, TILE], bf, tag="vt")
                nc.sync.dma_start(vt[:], vT_r[:, :, :, ts])
                nc.sync.dma_start(sT_sb[:, :, ts], sT_r[:, :, ts])
                for m in range(3):
                    ps = psA.tile([128, 3, TILE], f32, tag="eins")
                    for k in range(2):
                        lhsT = w0_sb[:, k * 384 + m * 128:k * 384 + (m + 1) * 128]
                        for c in range(3):
                            nc.tensor.matmul(ps[:, c, :], lhsT, vt[:, c, k, :],
                                             start=(k == 0), stop=(k == 1))
                    if m < 2:
                        sq = psq.tile([128, 3, TILE], f32, tag="sq")
                        nc.scalar.square(sq[:], ps[:])
                        with nc.allow_low_precision("norm^2 slab in bf16"):
                            nc.vector.tensor_reduce(
                                n0sq[:, m, ts], sq[:].rearrange("p c t -> p t c"),
                                axis=X, op=ALU.add)
                    else:
                        nc.scalar.copy(vecw0_sb[:, :, ts], ps[:])
            psA.release()

            # ================= P2: sqrt (in place) =================
            for t in range(T):
                ts = slice(t * TILE, (t + 1) * TILE)
                nc.scalar.sqrt(n0sq[:, :, ts], n0sq[:, :, ts])

            # ================= P3: gate0 MLP + gate mult =================
            with tc.tile_pool(name="ps3a", bufs=4, space="PSUM") as ps3a, \
                 tc.tile_pool(name="ps3b", bufs=2, space="PSUM") as ps3b:
                for t in range(T):
                    ts = slice(t * TILE, (t + 1) * TILE)
                    h = ph.tile([128, 4, TILE], bf, tag="h")
                    for m in range(4):
                        pm = ps3a.tile([128, TILE], f32, tag="m1")
                        for k in range(4):
                            lhsT = m1w_sb[:, k * 512 + m * 128:
                                          k * 512 + (m + 1) * 128]
                            rhs = (sT_sb[:, k, ts] if k < 2
                                   else n0sq[:, k - 2, ts])
                            nc.tensor.matmul(pm, lhsT, rhs,
                                             start=(k == 0), stop=(k == 3))
                        nc.scalar.activation(h[:, m, :], pm, AF.Silu,
                                             bias=m1b_sb[:, m:m + 1])
                    gate = ph.tile([128, TILE], bf, tag="gate")
                    for m2 in range(2):
                        p2 = ps3b.tile([128, TILE], f32, tag="m2")
                        for k in range(4):
                            lhsT = m2w_sb[:, k * 256 + m2 * 128:
                                          k * 256 + (m2 + 1) * 128]
                            nc.tensor.matmul(p2, lhsT, h[:, k, :],
                                             start=(k == 0), stop=(k == 3))
                        if m2 == 0:
                            nc.scalar.activation(q_sb[:, ts], p2, AF.Silu,
                                                 bias=m2b_sb[:, 0:1])
                        else:
                            nc.scalar.activation(gate[:], p2, AF.Identity,
                                                 bias=m2b_sb[:, 1:2])
                    for c in range(3):
                        nc.vector.tensor_mul(vecw0_sb[:, c, ts],
                                             vecw0_sb[:, c, ts], gate[:])

        # ================= P4: gate1 einsum + norm^2 + vec_w1^T ============
        with tc.tile_pool(name="slabD", bufs=1) as slabD:
            n1sq = slabD.tile([128, Nmax], bf)
            ps4a = tc.alloc_tile_pool(name="ps4a", bufs=2, space="PSUM")
            ps4b = tc.alloc_tile_pool(name="ps4b", bufs=2, space="PSUM")
            for t in range(T):
                ts = slice(t * TILE, (t + 1) * TILE)
                ps = ps4a.tile([128, 3, TILE], f32, tag="eins1")
                for c in range(3):
                    nc.tensor.matmul(ps[:, c, :], w1g_sb[:, 0:128],
                                     vecw0_sb[:, c, ts], start=True, stop=True)
                sq = psq.tile([128, 3, TILE], f32, tag="sq")
                nc.scalar.square(sq[:], ps[:])
                with nc.allow_low_precision("norm1^2 slab in bf16"):
                    nc.vector.tensor_reduce(
                        n1sq[:, ts], sq[:].rearrange("p c t -> p t c"),
                        axis=X, op=ALU.add)
                for ch in range(4):
                    g = t * 4 + ch
                    cs = slice(t * TILE + ch * PCH, t * TILE + (ch + 1) * PCH)
                    pw = ps4b.tile([128, 3], f32, tag="vw1")
                    for c in range(3):
                        nc.tensor.matmul(pw[:, c:c + 1],
                                         vecw0_sb[:, c, cs],
                                         w1g_sb[:, 128:129],
                                         start=True, stop=True)
                    nc.vector.tensor_copy(vw1T_sb[:, g, :], pw[:])
            ps4b.release()
            ps4a.release()

            # ================= P5: sqrt (in place) =================
            for t in range(T):
                ts = slice(t * TILE, (t + 1) * TILE)
                nc.scalar.sqrt(n1sq[:, ts], n1sq[:, ts])

            # ================= P6: gate1 MLP =================
            with tc.tile_pool(name="ps6a", bufs=2, space="PSUM") as ps6a, \
                 tc.tile_pool(name="ps6b", bufs=2, space="PSUM") as ps6b:
                for t in range(T):
                    ts = slice(t * TILE, (t + 1) * TILE)
                    h1 = ph.tile([128, 2, TILE], bf, tag="h1")
                    for m in range(2):
                        pm = ps6a.tile([128, TILE], f32, tag="m1g")
                        for k in range(2):
                            lhsT = m1wg_sb[:, k * 256 + m * 128:
                                           k * 256 + (m + 1) * 128]
                            rhs = q_sb[:, ts] if k == 0 else n1sq[:, ts]
                            nc.tensor.matmul(pm, lhsT, rhs,
                                             start=(k == 0), stop=(k == 1))
                        nc.scalar.activation(h1[:, m, :], pm, AF.Silu,
                                             bias=m1bg_sb[:, m:m + 1])
                    for ch in range(4):
                        g = t * 4 + ch
                        pe = ps6b.tile([128, 2], f32, tag="qg1")
                        for k in range(2):
                            nc.tensor.matmul(pe[:],
                                             h1[:, k, ch * PCH:(ch + 1) * PCH],
                                             m2wg_sb[:, k * 2:(k + 1) * 2],
                                             start=(k == 0), stop=(k == 1))
                        nc.vector.tensor_add(qg1T_sb[:, g, :], pe[:],
                                             brow2_sb[:])

        # ================= P7: node_mu + segment-sum matmul ===============
        with tc.tile_pool(name="psG", bufs=1, space="PSUM") as psG:
            psg = psG.tile([Gmax, 3], f32)
            for t in range(T):
                mcvt = psm.tile([128, 4, 3], f32, tag="mcv")
                nc.sync.dma_start(mcvt[:], mcv_r[:, t * 4:(t + 1) * 4, :])
                idst = psm.tile([128, 4], f32, tag="ids")
                nc.sync.dma_start(idst[:], ids_r[:, t * 4:(t + 1) * 4])
                for ch in range(4):
                    g = t * 4 + ch
                    tmp = psm.tile([128, 3], f32, tag="tmp")
                    nc.vector.tensor_scalar(tmp[:], mcvt[:, ch, :],
                                            qg1T_sb[:, g, 0:1], None, ALU.mult)
                    ndm = psm.tile([128, 3], bf, tag="ndm")
                    nc.vector.scalar_tensor_tensor(
                        ndm[:], vw1T_sb[:, g, :], qg1T_sb[:, g, 1:2], tmp[:],
                        ALU.mult, ALU.add)
                    oneh = psm.tile([128, Gmax], bf, tag="oneh")
                    nc.vector.tensor_scalar(oneh[:], iota_sb[:],
                                            idst[:, ch:ch + 1], None,
                                            ALU.is_equal)
                    nc.tensor.matmul(psg[:], oneh[:], ndm[:],
                                     start=(g == 0), stop=(g == NCH - 1))

            # ================= P8: final norms =================
            gm = psm.tile([Gmax, 3], f32, tag="gm")
            nc.vector.tensor_copy(gm[:], psg[:])
            gsq = psm.tile([Gmax, 3], f32, tag="gsq")
            nc.vector.tensor_mul(gsq[:], gm[:], gm[:])
            ss = psm.tile([Gmax, 1], f32, tag="ss")
            nc.vector.tensor_reduce(ss[:], gsq[:], axis=X, op=ALU.add)
            outsb = psm.tile([Gmax, 1], f32, tag="outsb")
            nc.scalar.sqrt(outsb[:], ss[:])
            nc.sync.dma_start(out_d[:], outsb[:])

        for p in (psm, ph, psq, pv, slabA, wp):
            p.release()

    nc.compile()
    return nc


def _prep(pos, mass_center, scaler, vector, batch_index, num_graphs,
          g0_w, g0_m1_w, g0_m1_b, g0_m2_w, g0_m2_b,
          g1_w, g1_m1_w, g1_m1_b, g1_m2_w, g1_m2_b):
    B = int(num_graphs)
    G = -(-B // N_CORES)
    bi = np.asarray(batch_index)
    edges = np.searchsorted(bi, np.arange(0, (N_CORES + 1) * G, G)[: N_CORES + 1],
                            side="left")
    n_k = np.diff(edges)
    Nmax = max(TILE, int(-(-max(n_k) // TILE) * TILE))

    # shared tensors
    shared = {
        "w0": _repack_k(np.asarray(g0_w)).astype(bfloat16),
        "m1w": _repack_k(np.asarray(g0_m1_w)).astype(bfloat16),
        "m2w": _repack_k(np.asarray(g0_m2_w)).astype(bfloat16),
        "w1g": np.ascontiguousarray(np.asarray(g1_w)).astype(bfloat16),
        "m1wg": _repack_k(np.asarray(g1_m1_w)).astype(bfloat16),
        "m2wg": _repack_k(np.asarray(g1_m2_w)).astype(bfloat16),
        "m1b": np.ascontiguousarray(
            np.asarray(g0_m1_b).reshape(4, 128).T).astype(np.float32),
        "m2b": np.ascontiguousarray(
            np.asarray(g0_m2_b).reshape(2, 128).T).astype(np.float32),
        "m1bg": np.ascontiguousarray(
            np.asarray(g1_m1_b).reshape(2, 128).T).astype(np.float32),
        "brow2": np.ascontiguousarray(
            np.broadcast_to(np.asarray(g1_m2_b, np.float32), (128, 2))),
        "iota": np.ascontiguousarray(
            np.broadcast_to(np.arange(G, dtype=np.float32), (128, G))),
    }

    pos = np.asarray(pos, np.float32)
    mc = np.asarray(mass_center, np.float32)
    scaler = np.asarray(scaler)
    vector = np.asarray(vector)
    mcv_full = pos - mc[bi]

    in_maps = []
    for k in range(N_CORES):
        s, e = int(edges[k]), int(edges[k + 1])
        n = e - s
        vT = np.zeros((3, IN_F, Nmax), bfloat16)
        vT[:, :, :n] = vector[s:e].astype(bfloat16).transpose(1, 2, 0)
        sTk = np.zeros((IN_F, Nmax), bfloat16)
        sTk[:, :n] = scaler[s:e].astype(bfloat16).T
        mcvk = np.zeros((Nmax, 3), np.float32)
        mcvk[:n] = mcv_full[s:e]
        idsk = np.full((Nmax,), G, np.float32)
        idsk[:n] = (bi[s:e] - k * G).astype(np.float32)
        in_maps.append({"vT": vT, "sT": sTk, "mcv": mcvk, "ids": idsk,
                        **shared})
    return in_maps, B, G, Nmax


def kernel(**inputs) -> np.ndarray:
    in_maps, B, G, Nmax = _prep(**inputs)
    key = (Nmax, G)
    if key not in _cache:
        _cache[key] = _build(Nmax, G)
    nc = _cache[key]
    res = run_bass_kernel_spmd(nc, in_maps, list(range(N_CORES)))
    out = np.empty((B, 1), np.float32)
    for k in range(N_CORES):
        lo = k * G
        hi = min(B, lo + G)
        if hi > lo:
            out[lo:hi] = res.results[k]["out"][: hi - lo]
    return out


# revision 26
# speedup vs baseline: 17.4963x; 17.4963x over previous
"""DipoleMomentDecoder Trainium2 kernel (8-core SPMD, full I/O).

Strategy
--------
Shard by graph: core k owns graphs [k*G, (k+1)*G), G = ceil(B/8).  batch_index
is sorted, so each core gets a contiguous token range (padded to Nmax tokens;
pad tokens carry graph id G so their one-hot row is zero and they drop out of
the segment sum).

On-chip layout is feature-major ([features(partitions), tokens(free)]): the two
gate blocks chain matmuls with no transposes.  The host pre-transposes/casts
the big activations (vector, scaler) to bf16 feature-major; all matmuls run
bf16 at full PE rate (plain fp32 matmul is 4x slower on TRN2).  PSUM stays
fp32; nonlinearities compute in fp32 on ACT.

ACT table sets: Sqrt and Silu live in different sets (~1.3us ACT_TABLE_LOAD
per switch), while Square/Copy/Identity are in every set.  The kernel is
phase-batched over two tile halves so only ~7 table loads happen, and while
ACT burns through one half's sqrt phase the PE runs the other half's dense
einsum/MLP matmuls (keeps the PE HAM clock-gate at full 2.4 GHz):
  P1(H1) einsum0+norm^2 > sqrt(H1) | P1(H2) > P3'(H1) [MLP0+silu, gate mult,
  lag-1: einsum1+norm1^2+token-major vec_w1 minis] > sqrt(H2)+sqrt1(H1) >
  P3'(H2) | P6(H1) [MLP1+silu, token-major q1/gate1 minis, node_mu via
  scalar_tensor_tensor, one-hot (is_equal vs iota) segment-sum matmul] >
  sqrt1(H2) > P6(H2) > final norms.
Explicit add_dep_helper edges pin the ACT phase order (the scheduler would
otherwise interleave sets) and chain sqrts in tile order so consumers of
tile 0 unblock first.

The segment sum is data-driven (one-hot built on device from graph ids), so
one SPMD program serves all 8 cores; per-core [G,1] norms are computed on
device and the host only concatenates 8 slices.  Measured on trn2 (N=65536,
B=512): ~353 us/core, abs-max relative error ~7e-4 vs the fp32 reference.
"""

import sys

for _p in ("/opt/trn_rl_repo", "/root/.axon_site/_ro/trn_rl_repo"):
    if _p not in sys.path:
        sys.path.insert(0, _p)

import numpy as np
from ml_dtypes import bfloat16

import concourse.bacc as bacc
from concourse.tile import add_dep_helper
import concourse.tile as tile
from concourse import mybir
from concourse.bass_utils import run_bass_kernel_spmd

F = mybir.dt
AF = mybir.ActivationFunctionType
ALU = mybir.AluOpType
X = mybir.AxisListType.X

IN_F = 256
HID = 128
N_CORES = 8
TILE = 512  # tokens per tile
PCH = 128  # tokens per partition-chunk

_cache = {}


def _repack_k(w):
    """[K, O] -> [128, (K//128)*O]; cols [k*O + m] = w[k*128 + p, m]."""
    k, o = w.shape
    kc = k // 128
    return np.ascontiguousarray(
        w.reshape(kc, 128, o).transpose(1, 0, 2).reshape(128, kc * o)
    )


def _build(Nmax, Gmax):
    T = Nmax // TILE
    NCH = Nmax // PCH
    nc = bacc.Bacc("TRN2", target_bir_lowering=False, debug=False,
                   num_devices=N_CORES)
    bf, f32 = F.bfloat16, F.float32

    # -------- DRAM I/O --------
    vT = nc.dram_tensor("vT", [3, IN_F, Nmax], bf, kind="ExternalInput")
    sT = nc.dram_tensor("sT", [IN_F, Nmax], bf, kind="ExternalInput")
    mcv = nc.dram_tensor("mcv", [Nmax, 3], f32, kind="ExternalInput")
    ids = nc.dram_tensor("ids", [Nmax], f32, kind="ExternalInput")
    w0_d = nc.dram_tensor("w0", [128, 2 * 384], bf, kind="ExternalInput")
    m1w_d = nc.dram_tensor("m1w", [128, 4 * 512], bf, kind="ExternalInput")
    m2w_d = nc.dram_tensor("m2w", [128, 4 * 256], bf, kind="ExternalInput")
    w1g_d = nc.dram_tensor("w1g", [128, 129], bf, kind="ExternalInput")
    m1wg_d = nc.dram_tensor("m1wg", [128, 2 * 256], bf, kind="ExternalInput")
    m2wg_d = nc.dram_tensor("m2wg", [128, 2 * 2], bf, kind="ExternalInput")
    m1b_d = nc.dram_tensor("m1b", [128, 4], f32, kind="ExternalInput")
    m2b_d = nc.dram_tensor("m2b", [128, 2], f32, kind="ExternalInput")
    m1bg_d = nc.dram_tensor("m1bg", [128, 2], f32, kind="ExternalInput")
    brow2_d = nc.dram_tensor("brow2", [128, 2], f32, kind="ExternalInput")
    iota_d = nc.dram_tensor("iota", [128, Gmax], f32, kind="ExternalInput")
    out_d = nc.dram_tensor("out", [Gmax, 1], f32, kind="ExternalOutput")

    vT_r = vT[:].rearrange("c (k p) n -> p c k n", p=128)  # [128,3,2,Nmax]
    sT_r = sT[:].rearrange("(k p) n -> p k n", p=128)      # [128,2,Nmax]
    mcv_r = mcv[:].rearrange("(u p) c -> p u c", p=128)    # [128,NCH,3]
    ids_r = ids[:].rearrange("(u p) -> p u", p=128)        # [128,NCH]

    act_ph = {i: [] for i in range(6)}  # ACT ops per table-set phase
    with tile.TileContext(nc) as tc:
        # ---- constants / weights (resident) ----
        wp = tc.alloc_tile_pool(name="wp", bufs=1)
        w0_sb = wp.tile([128, 2 * 384], bf)
        m1w_sb = wp.tile([128, 4 * 512], bf)
        m2w_sb = wp.tile([128, 4 * 256], bf)
        w1g_sb = wp.tile([128, 129], bf)
        m1wg_sb = wp.tile([128, 2 * 256], bf)
        m2wg_sb = wp.tile([128, 2 * 2], bf)
        m1b_sb = wp.tile([128, 4], f32)
        m2b_sb = wp.tile([128, 2], f32)
        m1bg_sb = wp.tile([128, 2], f32)
        brow2_sb = wp.tile([128, 2], f32)
        iota_sb = wp.tile([128, Gmax], f32)
        weight_dmas = [(w0_sb, w0_d), (m1w_sb, m1w_d), (m2w_sb, m2w_d),
                       (w1g_sb, w1g_d), (m1wg_sb, m1wg_d), (m2wg_sb, m2wg_d),
                       (m1b_sb, m1b_d), (m2b_sb, m2b_d), (m1bg_sb, m1bg_d),
                       (brow2_sb, brow2_d), (iota_sb, iota_d)]
        nc.sync.dma_start(w0_sb[:], w0_d[:])  # first einsum's weights first

        # ---- slabs that live (almost) the whole kernel ----
        slabA = tc.alloc_tile_pool(name="slabA", bufs=1)
        q_sb = slabA.tile([128, Nmax], bf)             # silu(gate0 s_out)
        vw1T_sb = slabA.tile([128, NCH, 3], bf)        # token-major vec_w1
        qg1T_sb = slabA.tile([128, NCH, 2], f32)       # token-major (q1,gate1)
        vecw0_sb = slabA.tile([128, 3, Nmax], bf)      # vec_w g0; becomes v1

        # ---- per-tile working pools (small, persistent) ----
        pv = tc.alloc_tile_pool(name="pv", bufs=2)     # vT tiles
        psq = tc.alloc_tile_pool(name="psq", bufs=1)   # squares
        ph = tc.alloc_tile_pool(name="ph", bufs=2)     # h / h1 / gate
        psm = tc.alloc_tile_pool(name="psm", bufs=8)   # small tail tiles

        # Two tile halves, pipelined: while ACT runs half-1's sqrt phase,
        # the PE runs half-2's dense einsums (Square/Copy/Identity live in
        # every ACT table set, so only Silu and Sqrt ops need ordering).
        # ACT set order: Q1 sqrt(H1) < S1 silu(P3'H1) < Q2 sqrt(H2+P5H1)
        #                < S2 silu(P3'H2+P6H1) < Q3 sqrt(P5H2) < S3 silu(P6H2)
        #                < Q4 sqrt(out)
        Th = (T + 1) // 2
        H1, H2 = list(range(Th)), list(range(Th, T))
        sqrt_ph = {i: [] for i in range(4)}
        silu_ph = {i: [] for i in range(3)}

        slabC = tc.alloc_tile_pool(name="slabC", bufs=1)
        sT_sb = slabC.tile([128, 2, Nmax], bf)
        n0sq = slabC.tile([128, 2, Nmax], bf)   # norm^2 then norm (in place)
        slabD = tc.alloc_tile_pool(name="slabD", bufs=1)
        n1sq = slabD.tile([128, Nmax], bf)

        big3 = tc.alloc_tile_pool(name="big3", bufs=1, space="PSUM")
        pmm = tc.alloc_tile_pool(name="pmm", bufs=3, space="PSUM")
        ptiny = tc.alloc_tile_pool(name="ptiny", bufs=1, space="PSUM")
        psG = tc.alloc_tile_pool(name="psG", bufs=1, space="PSUM")
        psg = psG.tile([Gmax, 3], f32)

        vts = {}
        for t in range(min(2, T)):  # prefetch before weight DMAs queue up
            ts = slice(t * TILE, (t + 1) * TILE)
            vt = pv.tile([128, 3, 2, TILE], bf, tag="vt", name=f"vtp_{t}")
            nc.sync.dma_start(vt[:], vT_r[:, :, :, ts])
            vts[t] = vt
        for sb, dd in weight_dmas[1:]:
            nc.sync.dma_start(sb[:], dd[:])

        def p1(tiles):
            # gate0 einsum + norm^2 (+ vec_w eviction)
            for t in tiles:
                ts = slice(t * TILE, (t + 1) * TILE)
                if t in vts:
                    vt = vts.pop(t)
                else:
                    vt = pv.tile([128, 3, 2, TILE], bf, tag="vt",
                                 name=f"vt_{t}")
                    nc.sync.dma_start(vt[:], vT_r[:, :, :, ts])
                nc.sync.dma_start(sT_sb[:, :, ts], sT_r[:, :, ts])
                for m in range(3):
                    ps = big3.tile([128, 3, TILE], f32, tag="big3",
                                   name=f"e0_{t}_{m}")
                    for k in range(2):
                        lhsT = w0_sb[:, k * 384 + m * 128:
                                     k * 384 + (m + 1) * 128]
                        for c in range(3):
                            nc.tensor.matmul(ps[:, c, :], lhsT,
                                             vt[:, c, k, :],
                                             start=(k == 0), stop=(k == 1))
                    if m < 2:
                        sq = psq.tile([128, 3, TILE], bf, tag="sq",
                                      name=f"sq0_{t}_{m}")
                        nc.scalar.square(sq[:], ps[:])
                        with nc.allow_low_precision("norm^2 slab in bf16"):
                            nc.vector.tensor_reduce(
                                n0sq[:, m, ts],
                                sq[:].rearrange("p c t -> p t c"),
                                axis=X, op=ALU.add)
                    else:
                        nc.scalar.copy(vecw0_sb[:, :, ts], ps[:])

        def p2(tiles, qi):
            for t in tiles:
                ts = slice(t * TILE, (t + 1) * TILE)
                sqrt_ph[qi].append(
                    nc.scalar.sqrt(n0sq[:, :, ts], n0sq[:, :, ts]))

        def p5(tiles, qi):
            for t in tiles:
                ts = slice(t * TILE, (t + 1) * TILE)
                sqrt_ph[qi].append(nc.scalar.sqrt(n1sq[:, ts], n1sq[:, ts]))

        def a3(t, si):
            # gate0 MLP + gate multiply
            ts = slice(t * TILE, (t + 1) * TILE)
            h = ph.tile([128, 4, TILE], bf, tag="h", name=f"h_{t}")
            for m in range(4):
                pm = pmm.tile([128, TILE], f32, tag="mm", name=f"pm_{t}_{m}")
                for k in range(4):
                    lhsT = m1w_sb[:, k * 512 + m * 128:k * 512 + (m + 1) * 128]
                    rhs = sT_sb[:, k, ts] if k < 2 else n0sq[:, k - 2, ts]
                    nc.tensor.matmul(pm, lhsT, rhs,
                                     start=(k == 0), stop=(k == 3))
                silu_ph[si].append(nc.scalar.activation(
                    h[:, m, :], pm, AF.Silu, bias=m1b_sb[:, m:m + 1]))
            gate = ph.tile([128, TILE], bf, tag="gate", name=f"gate_{t}")
            for m2 in range(2):
                p2t = pmm.tile([128, TILE], f32, tag="mm", name=f"p2_{t}_{m2}")
                for k in range(4):
                    lhsT = m2w_sb[:, k * 256 + m2 * 128:
                                  k * 256 + (m2 + 1) * 128]
                    nc.tensor.matmul(p2t, lhsT, h[:, k, :],
                                     start=(k == 0), stop=(k == 3))
                if m2 == 0:
                    silu_ph[si].append(nc.scalar.activation(
                        q_sb[:, ts], p2t, AF.Silu, bias=m2b_sb[:, 0:1]))
                else:
                    nc.scalar.activation(gate[:], p2t, AF.Identity,
                                         bias=m2b_sb[:, 1:2])
            for c in range(3):
                nc.vector.tensor_mul(vecw0_sb[:, c, ts],
                                     vecw0_sb[:, c, ts], gate[:])

        def b3(t):
            # gate1 einsum + norm1^2 + token-major vec_w1 minis
            ts = slice(t * TILE, (t + 1) * TILE)
            ps = big3.tile([128, 3, TILE], f32, tag="big3", name=f"e1_{t}")
            for c in range(3):
                nc.tensor.matmul(ps[:, c, :], w1g_sb[:, 0:128],
                                 vecw0_sb[:, c, ts], start=True, stop=True)
            sq = psq.tile([128, 3, TILE], bf, tag="sq", name=f"sq1_{t}")
            nc.scalar.square(sq[:], ps[:])
            with nc.allow_low_precision("norm1^2 slab in bf16"):
                nc.vector.tensor_reduce(
                    n1sq[:, ts], sq[:].rearrange("p c t -> p t c"),
                    axis=X, op=ALU.add)
            for ch in range(4):
                g = t * 4 + ch
                cs = slice(t * TILE + ch * PCH, t * TILE + (ch + 1) * PCH)
                pw = ptiny.tile([128, 3], f32, tag="tiny", name=f"pw_{t}_{ch}")
                for c in range(3):
                    nc.tensor.matmul(pw[:, c:c + 1], vecw0_sb[:, c, cs],
                                     w1g_sb[:, 128:129], start=True, stop=True)
                nc.vector.tensor_copy(vw1T_sb[:, g, :], pw[:])

        def p3(tiles, si):
            prev = None
            for t in tiles:
                a3(t, si)
                if prev is not None:
                    b3(prev)
                prev = t
            b3(prev)

        def a6(t, si):
            ts = slice(t * TILE, (t + 1) * TILE)
            h1 = ph.tile([128, 2, TILE], bf, tag="h1", bufs=3, name=f"h1_{t}")
            for m in range(2):
                pm = pmm.tile([128, TILE], f32, tag="mm", name=f"pm6_{t}_{m}")
                for k in range(2):
                    lhsT = m1wg_sb[:, k * 256 + m * 128:
                                   k * 256 + (m + 1) * 128]
                    rhs = q_sb[:, ts] if k == 0 else n1sq[:, ts]
                    nc.tensor.matmul(pm, lhsT, rhs,
                                     start=(k == 0), stop=(k == 1))
                silu_ph[si].append(nc.scalar.activation(
                    h1[:, m, :], pm, AF.Silu, bias=m1bg_sb[:, m:m + 1]))
            mcvt = psm.tile([128, 4, 3], f32, tag="mcv", name=f"mcv_{t}")
            nc.sync.dma_start(mcvt[:], mcv_r[:, t * 4:(t + 1) * 4, :])
            idst = psm.tile([128, 4], f32, tag="ids", name=f"ids_{t}")
            nc.sync.dma_start(idst[:], ids_r[:, t * 4:(t + 1) * 4])
            return h1, mcvt, idst

        def b6(t, h1, mcvt, idst):
            outs = []
            for ch in range(4):
                g = t * 4 + ch
                pe = ptiny.tile([128, 2], f32, tag="tiny", name=f"pe6_{t}_{ch}")
                for k in range(2):
                    nc.tensor.matmul(pe[:], h1[:, k, ch * PCH:(ch + 1) * PCH],
                                     m2wg_sb[:, k * 2:(k + 1) * 2],
                                     start=(k == 0), stop=(k == 1))
                nc.vector.tensor_add(qg1T_sb[:, g, :], pe[:], brow2_sb[:])
                tmp = psm.tile([128, 3], f32, tag="tmp", name=f"tmp_{t}_{ch}")
                nc.vector.tensor_scalar(tmp[:], mcvt[:, ch, :],
                                        qg1T_sb[:, g, 0:1], None, ALU.mult)
                ndm = psm.tile([128, 3], bf, tag="ndm", name=f"ndm_{t}_{ch}")
                nc.vector.scalar_tensor_tensor(
                    ndm[:], vw1T_sb[:, g, :], qg1T_sb[:, g, 1:2], tmp[:],
                    ALU.mult, ALU.add)
                oneh = psm.tile([128, Gmax], bf, tag="oneh",
                                name=f"oneh_{t}_{ch}")
                nc.vector.tensor_scalar(oneh[:], iota_sb[:],
                                        idst[:, ch:ch + 1], None, ALU.is_equal)
                outs.append((g, oneh, ndm))
            return outs

        def c6(outs):
            for g, oneh, ndm in outs:
                nc.tensor.matmul(psg[:], oneh[:], ndm[:],
                                 start=(g == 0), stop=(g == NCH - 1))

        def p6(tiles, si):
            As, Bs = {}, {}
            order = list(tiles)
            for i, t in enumerate(order):
                As[t] = a6(t, si)
                if i >= 1:
                    tb = order[i - 1]
                    Bs[tb] = b6(tb, *As.pop(tb))
                if i >= 2:
                    c6(Bs.pop(order[i - 2]))
            tl = order[-1]
            Bs[tl] = b6(tl, *As.pop(tl))
            if len(order) >= 2:
                c6(Bs.pop(order[-2]))
            c6(Bs.pop(tl))

        # ---------------- emission schedule ----------------
        p1(H1)
        p2(H1, 0)       # Q1
        p1(H2)          # dense PE work covering Q1's ACT barrier
        p3(H1, 0)       # S1
        p2(H2, 1)       # Q2a (chained first so P3'(H2) unblocks early)
        p5(H1, 1)       # Q2b
        p3(H2, 1)       # S2a
        p6(H1, 1)       # S2b
        p5(H2, 2)       # Q3
        p6(H2, 2)       # S3

        # ================= P8: final norms =================
        gm = psm.tile([Gmax, 3], f32, tag="gm")
        nc.vector.tensor_copy(gm[:], psg[:])
        gsq = psm.tile([Gmax, 3], f32, tag="gsq")
        nc.vector.tensor_mul(gsq[:], gm[:], gm[:])
        ss = psm.tile([Gmax, 1], f32, tag="ss")
        nc.vector.tensor_reduce(ss[:], gsq[:], axis=X, op=ALU.add)
        outsb = psm.tile([Gmax, 1], f32, tag="outsb")
        sqrt_ph[3].append(nc.scalar.sqrt(outsb[:], ss[:]))
        nc.sync.dma_start(out_d[:], outsb[:])

        # ---- ACT table-set ordering: Q1 < S1 < Q2 < S2 < Q3 < S3 < Q4 ----
        seq = [sqrt_ph[0], silu_ph[0], sqrt_ph[1], silu_ph[1],
               sqrt_ph[2], silu_ph[2], sqrt_ph[3]]
        for i in range(1, len(seq)):
            if seq[i - 1] and seq[i]:
                prev_last = seq[i - 1][-1]
                for inst in seq[i]:
                    add_dep_helper(inst.ins, prev_last.ins, sync=False,
                                   reason="ACT table-set phase order")
        for phl in sqrt_ph.values():  # consumers want tile order
            for j in range(1, len(phl)):
                add_dep_helper(phl[j].ins, phl[j - 1].ins, sync=False,
                               reason="sqrt tile order")

        for p in (psG, ptiny, pmm, big3, slabD, slabC):
            p.release()
        for p in (psm, ph, psq, pv, slabA, wp):
            p.release()

    nc.compile()
    return nc


def _prep(pos, mass_center, scaler, vector, batch_index, num_graphs,
          g0_w, g0_m1_w, g0_m1_b, g0_m2_w, g0_m2_b,
          g1_w, g1_m1_w, g1_m1_b, g1_m2_w, g1_m2_b):
    B = int(num_graphs)
    G = -(-B // N_CORES)
    bi = np.asarray(batch_index)
    edges = np.searchsorted(bi, np.arange(0, (N_CORES + 1) * G, G)[: N_CORES + 1],
                            side="left")
    n_k = np.diff(edges)
    Nmax = max(TILE, int(-(-max(n_k) // TILE) * TILE))

    # shared tensors
    shared = {
        "w0": _repack_k(np.asarray(g0_w)).astype(bfloat16),
        "m1w": _repack_k(np.asarray(g0_m1_w)).astype(bfloat16),
        "m2w": _repack_k(np.asarray(g0_m2_w)).astype(bfloat16),
        "w1g": np.ascontiguousarray(np.asarray(g1_w)).astype(bfloat16),
        "m1wg": _repack_k(np.asarray(g1_m1_w)).astype(bfloat16),
        "m2wg": _repack_k(np.asarray(g1_m2_w)).astype(bfloat16),
        "m1b": np.ascontiguousarray(
            np.asarray(g0_m1_b).reshape(4, 128).T).astype(np.float32),
        "m2b": np.ascontiguousarray(
            np.asarray(g0_m2_b).reshape(2, 128).T).astype(np.float32),
        "m1bg": np.ascontiguousarray(
            np.asarray(g1_m1_b).reshape(2, 128).T).astype(np.float32),
        "brow2": np.ascontiguousarray(
            np.broadcast_to(np.asarray(g1_m2_b, np.float32), (128, 2))),
        "iota": np.ascontiguousarray(
            np.broadcast_to(np.arange(G, dtype=np.float32), (128, G))),
    }

    pos = np.asarray(pos, np.float32)
    mc = np.asarray(mass_center, np.float32)
    scaler = np.asarray(scaler)
    vector = np.asarray(vector)
    mcv_full = pos - mc[bi]

    in_maps = []
    for k in range(N_CORES):
        s, e = int(edges[k]), int(edges[k + 1])
        n = e - s
        vT = np.zeros((3, IN_F, Nmax), bfloat16)
        vT[:, :, :n] = vector[s:e].astype(bfloat16).transpose(1, 2, 0)
        sTk = np.zeros((IN_F, Nmax), bfloat16)
        sTk[:, :n] = scaler[s:e].astype(bfloat16).T
        mcvk = np.zeros((Nmax, 3), np.float32)
        mcvk[:n] = mcv_full[s:e]
        idsk = np.full((Nmax,), G, np.float32)
        idsk[:n] = (bi[s:e] - k * G).astype(np.float32)
        in_maps.append({"vT": vT, "sT": sTk, "mcv": mcvk, "ids": idsk,
                        **shared})
    return in_maps, B, G, Nmax


def kernel(**inputs) -> np.ndarray:
    in_maps, B, G, Nmax = _prep(**inputs)
    key = (Nmax, G)
    if key not in _cache:
        _cache[key] = _build(Nmax, G)
    nc = _cache[key]
    res = run_bass_kernel_spmd(nc, in_maps, list(range(N_CORES)))
    out = np.empty((B, 1), np.float32)
    for k in range(N_CORES):
        lo = k * G
        hi = min(B, lo + G)
        if hi > lo:
            out[lo:hi] = res.results[k]["out"][: hi - lo]
    return out
